# revision 4
# baseline (speedup 1.0000x reference)
"""GAT layer kernel for Trainium2 (8 NeuronCores, SPMD row-sharded).

Math (reference):
    w_h  = input @ weights                      [N, 64]
    w_h1 = w_h @ a_values[:64]                  [N, 1]
    w_h2 = w_h @ a_values[64:]                  [N, 1]
    e    = leaky_relu(w_h1 + w_h2.T, 0.2)       [N, N]
    P    = where(adj != 0, exp(e), 0)           (softmax numerator; adj is 0/1)
    out  = elu((P @ w_h) / rowsum(P))           [N, 64]

Device-side factorization tricks:
  * exp(leaky_relu(x)) = max(exp(x), exp(0.2 x)) and exp(w_h1[i] + w_h2[j])
    = u[i] * v[j] with u = exp(w_h1), v = exp(w_h2) (rank-1), so the big
    [N, N] work needs no large activation passes -- only elementwise
    multiply/max on the vector engine.
  * Everything is computed transposed (P.T[j, i]) so the aggregation GEMM
    lhsT is P itself -- no transposes anywhere.
  * A ones-column appended to w_h makes the softmax denominators fall out
    of the same GEMM accumulation (PSUM row 64).

Each core owns 1024 output rows (i-shard); the j/contraction dim (8192)
is processed in 64 blocks of 128 partitions.
"""

import numpy as np
import ml_dtypes

N = 8192
IN_DIM = 256
OUT_DIM = 64
NCORES = 8
R = N // NCORES  # 1024 output rows per core
P = 128
NJB = N // P  # 64 j-blocks
ALPHA = 0.2

_NC_CACHE = {}


def build_nc(kdt_name="bfloat16"):
    """Build the (single-core SPMD) Bass program. kdt_name: compute dtype
    for the big [128, R] tiles: 'bfloat16' or 'float32'."""
    import concourse.bass as bass
    import concourse.mybir as mybir
    import concourse.tile as tile
    from concourse import bacc
    from concourse.bass import ts
    from contextlib import ExitStack

    f32 = mybir.dt.float32
    KDT = getattr(mybir.dt, kdt_name)
    AF = mybir.ActivationFunctionType
    OP = mybir.AluOpType

    nc = bacc.Bacc(trn_type="TRN2", target_bir_lowering=False, debug=False)

    adjt = nc.dram_tensor("adjt", [N, R], KDT, kind="ExternalInput").ap()
    inT = nc.dram_tensor("inT", [IN_DIM, N], f32, kind="ExternalInput").ap()
    inTmy = nc.dram_tensor("inTmy", [IN_DIM, R], f32, kind="ExternalInput").ap()
    waug = nc.dram_tensor("waug", [IN_DIM, 66], f32, kind="ExternalInput").ap()
    out = nc.dram_tensor("out", [OUT_DIM, R], f32, kind="ExternalOutput").ap()

    big_bufs = 3 if kdt_name == "bfloat16" else 2
    adj_bufs = 6 if kdt_name == "bfloat16" else 4

    with tile.TileContext(nc) as tc, ExitStack() as ctx:
        const = ctx.enter_context(tc.tile_pool(name="const", bufs=1))
        inT_sb = [const.tile([P, N], f32, tag=f"inT{k}", name=f"inT_sb{k}") for k in range(2)]
        inTmy_sb = [const.tile([P, R], f32, tag=f"inTmy{k}", name=f"inTmy_sb{k}") for k in range(2)]
        waug_sb = [const.tile([P, 66], f32, tag=f"waug{k}", name=f"waug_sb{k}") for k in range(2)]
        whaug = const.tile([P, NJB * (OUT_DIM + 1)], KDT, tag="whaug")
        vcol = const.tile([P, NJB], f32, tag="vcol")
        v2col = const.tile([P, NJB], f32, tag="v2col")
        urep = const.tile([P, R], KDT, tag="urep")
        u2rep = const.tile([P, R], KDT, tag="u2rep")
        ones = const.tile([1, P], f32, tag="ones")
        urow = const.tile([1, R], f32, tag="urow")
        u2row = const.tile([1, R], f32, tag="u2row")

        for k in range(2):
            nc.sync.dma_start(inT_sb[k][:], inT[ts(k, P), :])
            nc.sync.dma_start(inTmy_sb[k][:], inTmy[ts(k, P), :])
            nc.sync.dma_start(waug_sb[k][:], waug[ts(k, P), :])
        nc.vector.memset(ones[:], 1.0)
        # ones-column default: blocks overwrite cols 0:64 of each 65-stride
        nc.vector.memset(whaug[:], 1.0)

        wpsum = ctx.enter_context(tc.tile_pool(name="wpsum", bufs=2, space="PSUM"))
        misc_ps = ctx.enter_context(tc.tile_pool(name="miscps", bufs=1, space="PSUM"))

        # ---- w_h phase: w_h = input @ [weights | W@a1 | W@a2] --------------
        for jb in range(NJB):
            pw = wpsum.tile([P, 66], f32, tag="pw")
            nc.tensor.matmul(pw[:], inT_sb[0][:, ts(jb, P)], waug_sb[0][:],
                             start=True, stop=False)
            nc.tensor.matmul(pw[:], inT_sb[1][:, ts(jb, P)], waug_sb[1][:],
                             start=False, stop=True)
            # w_h block into the GEMM lhsT layout (cast to KDT)
            nc.vector.tensor_copy(whaug[:, jb * 65:jb * 65 + OUT_DIM],
                                  pw[:, 0:OUT_DIM])
            # v = exp(w_h2), v2 = exp(0.2 * w_h2) per-partition columns
            nc.scalar.activation(vcol[:, jb:jb + 1], pw[:, 65:66], AF.Exp)
            nc.scalar.activation(v2col[:, jb:jb + 1], pw[:, 65:66], AF.Exp,
                                 scale=ALPHA)

        # ---- w_h1 for my i-range -> u rows -> replicate across partitions --
        pw1 = misc_ps.tile([1, R], f32, tag="misc")
        for k in range(2):
            for h in range(R // 512):
                nc.tensor.matmul(pw1[:, ts(h, 512)], waug_sb[k][:, 64:65],
                                 inTmy_sb[k][:, ts(h, 512)],
                                 start=(k == 0), stop=(k == 1))
        nc.scalar.activation(urow[:], pw1[:], AF.Exp)
        nc.scalar.activation(u2row[:], pw1[:], AF.Exp, scale=ALPHA)
        for row, rep in ((urow, urep), (u2row, u2rep)):
            for h in range(R // 512):
                pr = misc_ps.tile([P, 512], f32, tag="misc")
                nc.tensor.matmul(pr[:], ones[:], row[:, ts(h, 512)],
                                 start=True, stop=True)
                nc.vector.tensor_copy(rep[:, ts(h, 512)], pr[:])

        # ---- main loop over j-blocks --------------------------------------
        adj_pool = ctx.enter_context(tc.tile_pool(name="adj", bufs=adj_bufs))
        t_pool = ctx.enter_context(tc.tile_pool(name="t", bufs=big_bufs))
        acc = ctx.enter_context(tc.tile_pool(name="acc", bufs=1, space="PSUM"))
        pres = acc.tile([OUT_DIM + 1, R], f32, tag="pres")

        for jb in range(NJB):
            adj_t = adj_pool.tile([P, R], KDT, tag="adj")
            nc.sync.dma_start(adj_t[:], adjt[ts(jb, P), :])
            # t2 = u2 * v2[jb]   (scalar engine, per-partition scale)
            t2 = t_pool.tile([P, R], KDT, tag="t2")
            nc.scalar.mul(t2[:], u2rep[:], v2col[:, jb:jb + 1])
            # m = max(u * v[jb], t2)   (fused scalar_tensor_tensor)
            m = t_pool.tile([P, R], KDT, tag="m")
            nc.vector.scalar_tensor_tensor(m[:], urep[:], vcol[:, jb:jb + 1],
                                           t2[:], OP.mult, OP.max)
            # P = m * adj  (masked softmax numerator, transposed)
            p_t = t_pool.tile([P, R], KDT, tag="p")
            nc.vector.tensor_tensor(p_t[:], m[:], adj_t[:], OP.mult)
            for h in range(R // 512):
                nc.tensor.matmul(pres[:, ts(h, 512)],
                                 whaug[:, jb * 65:(jb + 1) * 65],
                                 p_t[:, ts(h, 512)],
                                 start=(jb == 0), stop=(jb == NJB - 1))

        # ---- normalize + ELU + store --------------------------------------
        fin = ctx.enter_context(tc.tile_pool(name="fin", bufs=1))
        rcp = fin.tile([1, R], f32, tag="rcp")
        nc.vector.reciprocal(rcp[:], pres[OUT_DIM:OUT_DIM + 1, :])
        rr = acc.tile([OUT_DIM, R], f32, tag="rr")
        for h in range(R // 512):
            nc.tensor.matmul(rr[:, ts(h, 512)], ones[:, 0:OUT_DIM],
                             rcp[:, ts(h, 512)], start=True, stop=True)
        rrep = fin.tile([OUT_DIM, R], f32, tag="rrep")
        nc.vector.tensor_copy(rrep[:], rr[:])
        normd = fin.tile([OUT_DIM, R], f32, tag="normd")
        nc.vector.tensor_tensor(normd[:], pres[0:OUT_DIM, :], rrep[:], OP.mult)
        # elu(x) = max(x, exp(min(x, 0)) - 1)
        mn = fin.tile([OUT_DIM, R], f32, tag="mn")
        nc.vector.tensor_scalar(mn[:], normd[:], 0.0, None, OP.min)
        ex = fin.tile([OUT_DIM, R], f32, tag="ex")
        nc.scalar.activation(ex[:], mn[:], AF.Exp)
        outv = fin.tile([OUT_DIM, R], f32, tag="outv")
        nc.vector.scalar_tensor_tensor(outv[:], ex[:], -1.0, normd[:],
                                       OP.add, OP.max)
        nc.sync.dma_start(out[:], outv[:])

    nc.compile()
    return nc


def get_nc(kdt_name="bfloat16"):
    nc = _NC_CACHE.get(kdt_name)
    if nc is None:
        nc = _NC_CACHE[kdt_name] = build_nc(kdt_name)
    return nc


def prep_in_maps(input, adj_mat, weights, a_values, kdt_name="bfloat16"):
    np_kdt = np.dtype(ml_dtypes.bfloat16) if kdt_name == "bfloat16" else np.float32
    x = np.asarray(input, np.float32)
    adj = np.asarray(adj_mat, np.float32)
    w = np.asarray(weights, np.float32)
    a = np.asarray(a_values, np.float32)
    wa1 = w @ a[:OUT_DIM]
    wa2 = w @ a[OUT_DIM:]
    waug = np.concatenate([w, wa1, wa2], axis=1).astype(np.float32)
    inT = np.ascontiguousarray(x.T)
    adjT = adj.T
    in_maps = []
    for c in range(NCORES):
        sl = slice(c * R, (c + 1) * R)
        in_maps.append({
            "adjt": adjT[:, sl].astype(np_kdt),
            "inT": inT,
            "inTmy": np.ascontiguousarray(inT[:, sl]),
            "waug": waug,
        })
    return in_maps


def assemble_output(core_outs):
    outT = np.concatenate(core_outs, axis=1)  # [64, 8192]
    return np.ascontiguousarray(outT.T).astype(np.float32)


def kernel(input, adj_mat, weights, a_values, kdt_name="bfloat16",
           profile=False):
    from concourse.bass_utils import run_bass_kernel_spmd

    nc = get_nc(kdt_name)
    in_maps = prep_in_maps(input, adj_mat, weights, a_values, kdt_name)
    res = run_bass_kernel_spmd(nc, in_maps, list(range(NCORES)),
                               trace=profile)
    kernel.last_result = res
    if res.exec_time_ns is not None:
        print(f"HW exec time: {res.exec_time_ns} ns")
    return assemble_output([res.results[c]["out"] for c in range(NCORES)])


kernel.last_result = None
